# revision 25
# baseline (speedup 1.0000x reference)
"""Disentangled multi-head attention (DeBERTa-style) Trainium2 Bass kernel.

Full inputs in, full outputs out. Sharding: batch (B=8) across 8 cores, data
parallel; each core computes all H=8 heads for its batch element.

Math (per batch b):
  q,k,v = x@W? + b?                                   [S, D]
  rel_emb[i,j] = rel_tab[j-i+511]  (Toeplitz: only 1023 distinct rows)
  P_k = rel_tab@Wpk + bpk ; P_q = rel_tab@Wpq + bpq   [1023, D]
  c2c[i,j] = q_i . k_j
  c2p[i,j] = q_i . P_k[j-i+511]  = qP[i, j-i+511],    qP  = q @ P_k^T
  p2c[i,j] = k_j . P_q[j-i+511]  = kPf[j, i-j+511],   kPf = k @ P_qflip^T
  out = softmax((c2c+c2p+p2c)/sqrt(3*64)) @ v ; y = out@Wo + bo

Kernel works in transposed-logits layout logitsT[j, i]:
  c2cT  : matmul(lhsT=khT_chunk, rhs=qhT)
  c2pT  : diag-DMA qP rows (per-partition shifted slice) then PE-transpose
  p2cT  : diag-DMA kPf rows directly (already [j, i])
  softmax: exp on ACT; denominator via ones-column in the AV matmul;
  normalize after AV.

Perf structure (cost-model driven):
  - inputs loaded with one packed DMA per tensor, issued from the Pool
    engine's SWDGE path so descriptor-gen doesn't serialize on HWDGE
  - qP/kPf windows computed as 384+256 col matmuls (both >=256 so fp32r
    streams 1 cyc/row; 512+128 pays 4x on the 128)
  - the 4 per-chunk diagonal reads of each pipeline merge into ONE 3-D-AP
    DMA (HWDGE descriptor-gen is ~630ns per DMA instruction, serialized)
  - v is evicted directly in ones-augmented per-head layout; odd heads use
    a reversed [1|v] layout so their AV matmul lands on partitions 63..127,
    letting head pairs share one [128, S] outT tile
  - phase C contracts head PAIRS in single K=128 matmuls (16 not 32)
"""

import math
import os
import sys
import threading

import numpy as np
import ml_dtypes

for _p in ("/opt/trn_rl_repo",):
    if _p not in sys.path and os.path.isdir(_p):
        sys.path.insert(0, _p)

import concourse.bacc as bacc
import concourse.bass as bass
import concourse.mybir as mybir
import concourse.tile as tile
from concourse.ap import AP
from concourse.bass_utils import run_bass_kernel_spmd
from concourse.masks import make_identity

S = 512
D = 512
H = 8
DH = 64
L = 512
W = 2 * L - 1  # 1023
WP = 1024  # padded so fp32r matmuls keep even 512-wide moving dims
WIN = 640  # 639-wide diag window, rounded up
NCORES = 8
SCALE = 1.0 / math.sqrt(3.0 * DH)

F32 = mybir.dt.float32
F32R = mybir.dt.float32r
BF16 = mybir.dt.bfloat16
MM_DT = F32R


def _merged_diag_ap(t, col0, nchunks, chunk_stride, nrows, ncols):
    """Per-partition shifted read over nchunks windows packed in one tile:
    out[p, c*ncols + j] = t[p, c*chunk_stride + col0 - p + j]."""
    rs = t.ap[0][0]
    return AP(
        t.tensor,
        t.offset + col0,
        [[rs - 1, nrows], [chunk_stride, nchunks], [1, ncols]],
    )


def _rev_ap(t, ncols):
    """Free-dim reversed view of a [P, ncols] tile/psum AP."""
    rs = t.ap[0][0]
    return AP(t.tensor, t.offset + ncols - 1, [[rs, t.shape[0]], [-1, ncols]])


def build_program():
    nc = bacc.Bacc(trn_type="TRN2")

    x = nc.dram_tensor("x", [S, D], MM_DT, kind="ExternalInput")
    Wq = nc.dram_tensor("Wq", [D, D], BF16, kind="ExternalInput")
    bq = nc.dram_tensor("bq", [D], F32, kind="ExternalInput")
    Wk = nc.dram_tensor("Wk", [D, D], BF16, kind="ExternalInput")
    bk = nc.dram_tensor("bk", [D], F32, kind="ExternalInput")
    Wv = nc.dram_tensor("Wv", [D, D], MM_DT, kind="ExternalInput")
    bv = nc.dram_tensor("bv", [D], F32, kind="ExternalInput")
    rel_tab = nc.dram_tensor("rel_tab", [W, D], MM_DT, kind="ExternalInput")
    Wpk = nc.dram_tensor("Wpk", [D, D], BF16, kind="ExternalInput")
    bpk = nc.dram_tensor("bpk", [D], F32, kind="ExternalInput")
    Wpq = nc.dram_tensor("Wpq", [D, D], BF16, kind="ExternalInput")
    bpq = nc.dram_tensor("bpq", [D], F32, kind="ExternalInput")
    Wo = nc.dram_tensor("Wo", [D, D], MM_DT, kind="ExternalInput")
    bo = nc.dram_tensor("bo", [D], F32, kind="ExternalInput")
    y = nc.dram_tensor("y", [S, D], F32, kind="ExternalOutput")

    with tile.TileContext(nc) as tc:
        with (
            tc.tile_pool(name="const", bufs=1) as constp,
            tc.tile_pool(name="persist", bufs=1) as persist,
        ):
            ident = constp.tile([128, 128], F32, name="ident")
            make_identity(nc, ident)
            ident_r = constp.tile([128, 128], MM_DT, name="ident_r")
            nc.scalar.copy(ident_r[:], ident[:])
            ident_b = constp.tile([128, 128], BF16, name="ident_b")
            nc.scalar.copy(ident_b[:], ident[:])

            # =========================== phase A ===========================
            with (
                tc.tile_pool(name="wload", bufs=1) as wload,
                tc.tile_pool(name="ps_xt", bufs=2, space="PSUM") as ps_xt,
                tc.tile_pool(name="ps_rt", bufs=2, space="PSUM") as ps_rt,
                tc.tile_pool(name="ps_pj", bufs=2, space="PSUM") as ps_pj,
            ):

                def load_packed(dram, nrows, name, eng, dt=MM_DT, pool=None):
                    """One big DMA: [nrows, D] row chunks packed side by side
                    in the free dim; chunk c = tile[:, c*D:(c+1)*D]."""
                    nch = (nrows + 127) // 128
                    t = (pool or wload).tile([128, nch * D], dt, name=name)
                    full = nrows // 128
                    flat = dram[:, :].rearrange("a b -> (a b)")
                    rs = t.ap[0][0]
                    if full:
                        eng.dma_start(
                            AP(t.tensor, t.offset, [[rs, 128], [D, full], [1, D]]),
                            AP(flat.tensor, 0, [[D, 128], [128 * D, full], [1, D]]),
                        )
                    if full < nch:  # remainder rows
                        p = nrows - full * 128
                        eng.dma_start(
                            t[:p, full * D : full * D + D],
                            dram[full * 128 : nrows, :],
                        )
                    return [t[:, c * D : (c + 1) * D] for c in range(nch)]

                # x first (critical path to xT), weights on the Pool/SWDGE
                # path so HWDGE stays free for latency-critical DMAs.
                x_t = load_packed(x, S, "x", eng=nc.sync)
                Wq_t = load_packed(Wq, D, "Wq", eng=nc.gpsimd, dt=BF16)
                Wk_t = load_packed(Wk, D, "Wk", eng=nc.gpsimd, dt=BF16)
                rel_t = load_packed(rel_tab, W, "rel", eng=nc.gpsimd)
                Wpk_t = load_packed(Wpk, D, "Wpk", eng=nc.gpsimd, dt=BF16)
                Wpq_t = load_packed(Wpq, D, "Wpq", eng=nc.gpsimd, dt=BF16)
                Wv_t = load_packed(Wv, D, "Wv", eng=nc.gpsimd)
                # Wo chunk c holds rows c*128..c*128+127 = head pair c;
                # lives in persist (read in phase C, after wload closes)
                Wo_h2 = load_packed(Wo, D, "Wo", eng=nc.gpsimd, pool=persist)

                bv_bc = constp.tile([128, D], F32, name="bv_bc")
                nc.sync.dma_start(bv_bc[:], AP(bv[:].tensor, 0, [[0, 128], [1, D]]))
                bo_bc = constp.tile([128, D], F32, name="bo_bc")
                nc.sync.dma_start(bo_bc[:], AP(bo[:].tensor, 0, [[0, 128], [1, D]]))

                def load_bias_cols(dram, name):
                    t = constp.tile([128, 4], F32, name=name)
                    rs = t.ap[0][0]
                    nc.sync.dma_start(
                        AP(t.tensor, t.offset, [[rs, 128], [1, 4], [1, 1]]),
                        AP(dram[:].tensor, 0, [[1, 128], [128, 4], [1, 1]]),
                    )
                    return [t[:, c : c + 1] for c in range(4)]

                bq_t = load_bias_cols(bq, "bq")
                bk_t = load_bias_cols(bk, "bk")
                bpk_t = load_bias_cols(bpk, "bpk")
                bpq_t = load_bias_cols(bpq, "bpq")

                # ---- xT via PE transpose; evicted twice: fp32r for the
                # v projection, bf16 for the q/k projections ----
                xT_t, xTb_t = [], []
                for ec in range(4):
                    ps = ps_xt.tile([128, S], F32, name="ps_xtt", tag="ps_xtt")
                    for sc in range(4):
                        nc.tensor.matmul(
                            ps[:, sc * 128 : (sc + 1) * 128].bitcast(MM_DT),
                            x_t[sc][:, ec * 128 : (ec + 1) * 128],
                            ident_r[:],
                            is_transpose=True,
                            start=(sc == 0),
                            stop=(sc == 3),
                        )
                    t = wload.tile([128, S], MM_DT, name=f"xT{ec}")
                    nc.scalar.copy(t[:], ps[:])
                    xT_t.append(t)
                    tb = wload.tile([128, S], BF16, name=f"xTb{ec}")
                    nc.vector.tensor_copy(tb[:], ps[:])
                    xTb_t.append(tb)

                # ---- qT, kT (per-partition bias) ----
                def proj_T(W_t, b_t, name):
                    out = []
                    for dcc in range(4):
                        ps = ps_pj.tile([128, S], F32, name="ps_prj", tag="ps_prj")
                        for ec in range(4):
                            nc.tensor.matmul(
                                ps[:],
                                W_t[ec][:, dcc * 128 : (dcc + 1) * 128],
                                xTb_t[ec][:],
                                start=(ec == 0),
                                stop=(ec == 3),
                            )
                        t = persist.tile([128, S], BF16, name=f"{name}{dcc}")
                        nc.scalar.activation(
                            t[:],
                            ps[:],
                            mybir.ActivationFunctionType.Identity,
                            bias=b_t[dcc],
                        )
                        out.append(t)
                    return out

                qT_t = proj_T(Wq_t, bq_t, "qT")
                kT_t = proj_T(Wk_t, bk_t, "kT")

                # ---- v straight into ones-augmented per-head layout ----
                # per sc: tile [128, 8*65]; head h: cols [65h, 65h+64) = v,
                # col 65h+64 = 1.0.  AV lhsT = tile[:, 65h:65h+65].
                vh_all = []
                for sc in range(4):
                    ps = ps_pj.tile([128, D], F32, name="ps_vv", tag="ps_prj")
                    for ec in range(4):
                        nc.tensor.matmul(
                            ps[:],
                            xT_t[ec][:, sc * 128 : (sc + 1) * 128],
                            Wv_t[ec][:],
                            start=(ec == 0),
                            stop=(ec == 3),
                        )
                    va = persist.tile([128, H * (DH + 1)], MM_DT, name=f"vaug{sc}")
                    rs = va.ap[0][0]
                    nc.vector.tensor_add(
                        AP(va.tensor, va.offset, [[rs, 128], [DH + 1, H], [1, DH]]),
                        ps[:],
                        bv_bc[:],
                    )
                    nc.vector.memset(
                        AP(va.tensor, va.offset + DH, [[rs, 128], [DH + 1, H], [1, 1]]).bitcast(F32),
                        1.0,
                    )
                    vh_all.append(va)

                def vaug_h(h, sc):
                    return vh_all[sc][:, h * (DH + 1) : (h + 1) * (DH + 1)]

                # ---- rel_tabT via PE transpose: [512, 1023] ----
                relT_t = []
                for dc in range(4):
                    ps = ps_rt.tile([128, WP], F32, name="ps_rtt", tag="ps_rtt")
                    for rc in range(8):
                        # last chunk has 127 valid rows; transpose all 128 --
                        # the garbage column lands in the pad col 1023, which
                        # the eviction below never reads.
                        nc.tensor.matmul(
                            ps[:, rc * 128 : rc * 128 + 128].bitcast(MM_DT),
                            rel_t[rc][:, dc * 128 : (dc + 1) * 128],
                            ident_r[:],
                            is_transpose=True,
                            start=(rc % 4 == 0),
                            stop=(rc % 4 == 3),
                        )
                    t = wload.tile([128, WP], BF16, name=f"relT{dc}")
                    if dc % 2 == 0:
                        nc.vector.tensor_copy(t[:, 0:W], ps[:, 0:W])
                    else:
                        nc.scalar.copy(t[:, 0:W], ps[:, 0:W])
                    nc.vector.memset(t[:, W:WP], 0.0)
                    relT_t.append(t)

                # ---- P_kT [512, 1024] and P_qT flipped ----
                def posproj_chunk(W_t, b_t, name, flip, dcc):
                    ps = ps_rt.tile([128, WP], F32, name="ps_pp", tag="ps_rtt")
                    for n0 in (0, 512):
                        for ec in range(4):
                            nc.tensor.matmul(
                                ps[:, n0 : n0 + 512],
                                W_t[ec][:, dcc * 128 : (dcc + 1) * 128],
                                relT_t[ec][:, n0 : n0 + 512],
                                start=(ec == 0),
                                stop=(ec == 3),
                            )
                    t = persist.tile([128, WP], BF16, name=f"{name}{dcc}")
                    if flip:
                        nc.scalar.activation(
                            t[:, 0:W],
                            _rev_ap(ps, W),
                            mybir.ActivationFunctionType.Identity,
                            bias=b_t[dcc],
                        )
                        nc.vector.memset(t[:, W:WP], 0.0)
                    else:
                        nc.scalar.activation(
                            t[:],
                            ps[:],
                            mybir.ActivationFunctionType.Identity,
                            bias=b_t[dcc],
                        )
                    return t

                PkT_t, PqTf_t = [], []
                for dcc in range(4):
                    PkT_t.append(
                        posproj_chunk(Wpk_t, bpk_t, "PkT", False, dcc)
                    )
                    PqTf_t.append(
                        posproj_chunk(Wpq_t, bpq_t, "PqTf", True, dcc)
                    )

            # =========================== phase B ===========================
            # Heads in pairs: even head on partitions 0-63, odd on 64-127.
            with (
                tc.tile_pool(name="hwork", bufs=2) as hwork,
                tc.tile_pool(name="ps_qp", bufs=3, space="PSUM") as ps_qp,
                tc.tile_pool(name="ps_lg", bufs=2, space="PSUM") as ps_lg,
                tc.tile_pool(name="ps_av", bufs=2, space="PSUM") as ps_av,
                tc.tile_pool(name="ps_y", bufs=1, space="PSUM") as ps_yp,
            ):

                def qp_pipeline(thT, PhT, tag, dt):
                    """qP/kPf window -> evict -> ONE merged diag read.
                    Window of qP row-chunk ic is the 640 cols
                    [384-i0, 1024-i0); computed as 384+256 col matmuls (both
                    >=256 keeps fp32r at 1 cyc/row).  All 4 chunks evict into
                    one [128, 4*640] tile; a single 3-D-AP DMA pulls the four
                    diagonals at once.  Returns the [128, 4*512] diag tile."""
                    sb = hwork.tile([128, 4 * WIN], dt, name=f"{tag}sb", bufs=3)
                    for ic in range(4):
                        i0 = ic * 128
                        pa = ps_qp.tile([128, 384], F32, name="ps_qpa", tag="ps_qp")
                        nc.tensor.matmul(
                            pa[:],
                            thT[:, i0 : i0 + 128],
                            PhT[:, 384 - i0 : 768 - i0],
                        )
                        pb = ps_qp.tile([128, 256], F32, name="ps_qpb", tag="ps_qp")
                        nc.tensor.matmul(
                            pb[:],
                            thT[:, i0 : i0 + 128],
                            PhT[:, 768 - i0 : 1024 - i0],
                        )
                        c0 = ic * WIN
                        if ic % 2 == 0:
                            nc.vector.tensor_copy(sb[:, c0 : c0 + 384], pa[:])
                            nc.scalar.copy(sb[:, c0 + 384 : c0 + 640], pb[:])
                        else:
                            nc.scalar.copy(sb[:, c0 : c0 + 384], pa[:])
                            nc.vector.tensor_copy(sb[:, c0 + 384 : c0 + 640], pb[:])
                    dg = hwork.tile([128, 4 * S], dt, name=f"{tag}dg", bufs=3)
                    nc.sync.dma_start(dg[:], _merged_diag_ap(sb, 127, 4, WIN, 128, S))
                    return dg

                outT_pair = [
                    persist.tile([128, S], MM_DT, name=f"outT{p}") for p in range(4)
                ]

                def head_views(h):
                    dc, hs = h // 2, (h % 2) * DH
                    return (
                        qT_t[dc][hs : hs + DH, :],
                        kT_t[dc][hs : hs + DH, :],
                        PkT_t[dc][hs : hs + DH, :],
                        PqTf_t[dc][hs : hs + DH, :],
                    )

                def emit_pipes(h):
                    qhT, khT, PkhT, PqhTf = head_views(h)
                    c2p = qp_pipeline(qhT, PkhT, "qp", MM_DT)
                    p2cT = qp_pipeline(khT, PqhTf, "kp", BF16)
                    return c2p, p2cT

                # ysb pre-loaded with bo; per-pair phase C accumulates
                # into it via Pool adds as soon as each pair completes
                ysb = hwork.tile([128, 4 * D], F32, name="ysb", bufs=1)
                for sc in range(4):
                    nc.vector.tensor_copy(ysb[:, sc * D : (sc + 1) * D], bo_bc[:])

                def emit_pairC_item(p, sc):
                    ps = ps_yp.tile([128, D], F32, name="ps_y", tag="ps_y")
                    nc.tensor.matmul(
                        ps[:],
                        outT_pair[p][:, sc * 128 : (sc + 1) * 128],
                        Wo_h2[p][:],
                    )
                    nc.vector.tensor_tensor(
                        ysb[:, sc * D : (sc + 1) * D],
                        ysb[:, sc * D : (sc + 1) * D],
                        ps[:],
                        op=mybir.AluOpType.add,
                    )
                    if p == 3:
                        nc.sync.dma_start(
                            y[sc * 128 : (sc + 1) * 128, :],
                            ysb[:, sc * D : (sc + 1) * D],
                        )

                horder = [0, 1, 3, 2, 5, 4, 7, 6]  # end on an even head
                pipes = {horder[0]: emit_pipes(horder[0]),
                         horder[1]: emit_pipes(horder[1])}
                pendingC = []
                for hi, h in enumerate(horder):
                    qhT, khT, PkhT, PqhTf = head_views(h)
                    if hi + 2 < H:
                        pipes[horder[hi + 2]] = emit_pipes(horder[hi + 2])
                    c2p, p2cT = pipes.pop(h)

                    ex = []
                    for jc in range(4):
                        ps = ps_lg.tile([128, S], F32, name="ps_lg", tag="ps_lg")
                        nc.tensor.matmul(
                            ps[:],
                            khT[:, jc * 128 : (jc + 1) * 128],
                            qhT[:],
                            start=True,
                            stop=False,
                        )
                        for ic in range(4):
                            nc.tensor.matmul(
                                ps[:, ic * 128 : (ic + 1) * 128].bitcast(MM_DT),
                                c2p[:, ic * S + jc * 128 : ic * S + (jc + 1) * 128],
                                ident_r[:],
                                is_transpose=True,
                                start=False,
                                stop=False,
                            )
                        nc.tensor.matmul(
                            ps[:],
                            ident_b[:],
                            p2cT[:, jc * S : (jc + 1) * S],
                            start=False,
                            stop=True,
                        )
                        et = hwork.tile([128, S], MM_DT, name=f"ex{jc}", bufs=3)
                        nc.scalar.activation(
                            et[:],
                            ps[:],
                            mybir.ActivationFunctionType.Exp,
                            scale=SCALE,
                        )
                        ex.append(et[:])
                        if pendingC:
                            emit_pairC_item(*pendingC.pop(0))

                    # AV with ones-row: denominator lands in psum row 64.
                    avt = ps_av.tile([DH + 1, S], F32, name="ps_avt", tag="ps_avt")
                    for jc in range(4):
                        nc.tensor.matmul(
                            avt[:],
                            vaug_h(h, jc),
                            ex[jc],
                            start=(jc == 0),
                            stop=(jc == 3),
                        )
                    zr = hwork.tile([DH + 1, S], F32, name="zrow", bufs=2)
                    nc.vector.reciprocal(zr[DH : DH + 1, :], avt[DH : DH + 1, :])
                    rbc = hwork.tile([DH, S], F32, name="rbc", bufs=2)
                    rs_zr = zr.ap[0][0]
                    nc.gpsimd.partition_broadcast(
                        rbc[:],
                        AP(zr.tensor, zr.offset + DH * rs_zr, [[rs_zr, 1], [1, S]]),
                    )
                    if h % 2 == 0:
                        nc.vector.tensor_mul(
                            outT_pair[h // 2][0:DH, :], avt[0:DH, :], rbc[:]
                        )
                    else:
                        # odd head: normalize into a staging tile, then DMA
                        # into partitions 64..127 of the pair tile (engines
                        # cannot shift partitions; DMA can)
                        omt = hwork.tile([DH, S], MM_DT, name="otmp", bufs=2)
                        nc.vector.tensor_mul(omt[:], avt[0:DH, :], rbc[:])
                        nc.sync.dma_start(
                            outT_pair[h // 2][DH:128, :], omt[:]
                        )
                    if hi % 2 == 1:
                        pendingC.extend((h // 2, sc) for sc in range(4))

                while pendingC:
                    emit_pairC_item(*pendingC.pop(0))

    nc.compile()
    return nc


_cache_lock = threading.Lock()
_cached_nc = None


def _get_program():
    global _cached_nc
    with _cache_lock:
        if _cached_nc is None:
            _cached_nc = build_program()
    return _cached_nc


def kernel(**inputs):
    x = np.ascontiguousarray(np.asarray(inputs["x"], dtype=np.float32))
    B = x.shape[0]
    assert x.shape == (B, S, D)

    weights = {
        k: np.ascontiguousarray(np.asarray(inputs[k], dtype=np.float32))
        for k in (
            "Wq", "bq", "Wk", "bk", "Wv", "bv",
            "rel_tab", "bpk", "bpq", "Wo", "bo",
        )
    }
    for k in ("Wpk", "Wpq", "Wq", "Wk"):  # device expects bf16 here
        weights[k] = np.ascontiguousarray(
            np.asarray(inputs[k], dtype=np.float32).astype(ml_dtypes.bfloat16)
        )

    nc = _get_program()
    in_maps = [{"x": x[c], **weights} for c in range(NCORES)]
    res = run_bass_kernel_spmd(nc, in_maps, core_ids=list(range(NCORES)))
    out = np.stack([res.results[c]["y"] for c in range(NCORES)], axis=0)
    return out.astype(np.float32)


if __name__ == "__main__":
    rng = np.random.default_rng(0)
    ins = {
        "x": rng.standard_normal((NCORES, S, D), dtype=np.float32),
        "rel_tab": rng.standard_normal((W, D), dtype=np.float32),
    }
    for nm in ("Wq", "Wk", "Wv", "Wpk", "Wpq", "Wo"):
        ins[nm] = rng.standard_normal((D, D), dtype=np.float32) * 0.04
    for nm in ("bq", "bk", "bv", "bpk", "bpq", "bo"):
        ins[nm] = rng.standard_normal(D).astype(np.float32) * 0.01
    out = kernel(**ins)
    print("ran:", out.shape, out.dtype, np.abs(out).max())


# revision 27
# speedup vs baseline: 1.0254x; 1.0254x over previous
"""Disentangled multi-head attention (DeBERTa-style) Trainium2 Bass kernel.

Full inputs in, full outputs out. Sharding: batch (B=8) across 8 cores, data
parallel; each core computes all H=8 heads for its batch element.

Math (per batch b):
  q,k,v = x@W? + b?                                   [S, D]
  rel_emb[i,j] = rel_tab[j-i+511]  (Toeplitz: only 1023 distinct rows)
  P_k = rel_tab@Wpk + bpk ; P_q = rel_tab@Wpq + bpq   [1023, D]
  c2c[i,j] = q_i . k_j
  c2p[i,j] = q_i . P_k[j-i+511]  = qP[i, j-i+511],    qP  = q @ P_k^T
  p2c[i,j] = k_j . P_q[j-i+511]  = kPf[j, i-j+511],   kPf = k @ P_qflip^T
  out = softmax((c2c+c2p+p2c)/sqrt(3*64)) @ v ; y = out@Wo + bo

Kernel works in transposed-logits layout logitsT[j, i]:
  c2cT  : matmul(lhsT=khT_chunk, rhs=qhT)
  c2pT  : diag-DMA qP rows (per-partition shifted slice) then PE-transpose
  p2cT  : diag-DMA kPf rows directly (already [j, i])
  softmax: exp on ACT; denominator via ones-column in the AV matmul;
  normalize after AV.

Perf structure (cost-model driven):
  - inputs loaded with one packed DMA per tensor, issued from the Pool
    engine's SWDGE path so descriptor-gen doesn't serialize on HWDGE
  - qP/kPf windows computed as 384+256 col matmuls (both >=256 so fp32r
    streams 1 cyc/row; 512+128 pays 4x on the 128)
  - the 4 per-chunk diagonal reads of each pipeline merge into ONE 3-D-AP
    DMA (HWDGE descriptor-gen is ~630ns per DMA instruction, serialized)
  - v is evicted directly in ones-augmented per-head layout; odd heads use
    a reversed [1|v] layout so their AV matmul lands on partitions 63..127,
    letting head pairs share one [128, S] outT tile
  - phase C contracts head PAIRS in single K=128 matmuls (16 not 32)
"""

import math
import os
import sys
import threading

import numpy as np
import ml_dtypes

for _p in ("/opt/trn_rl_repo",):
    if _p not in sys.path and os.path.isdir(_p):
        sys.path.insert(0, _p)

import concourse.bacc as bacc
import concourse.bass as bass
import concourse.mybir as mybir
import concourse.tile as tile
from concourse.ap import AP
from concourse.bass_utils import run_bass_kernel_spmd
from concourse.masks import make_identity

S = 512
D = 512
H = 8
DH = 64
L = 512
W = 2 * L - 1  # 1023
WP = 1024  # padded so fp32r matmuls keep even 512-wide moving dims
WIN = 640  # 639-wide diag window, rounded up
NCORES = 8
SCALE = 1.0 / math.sqrt(3.0 * DH)

F32 = mybir.dt.float32
F32R = mybir.dt.float32r
BF16 = mybir.dt.bfloat16
MM_DT = F32R


def _merged_diag_ap(t, col0, nchunks, chunk_stride, nrows, ncols):
    """Per-partition shifted read over nchunks windows packed in one tile:
    out[p, c*ncols + j] = t[p, c*chunk_stride + col0 - p + j]."""
    rs = t.ap[0][0]
    return AP(
        t.tensor,
        t.offset + col0,
        [[rs - 1, nrows], [chunk_stride, nchunks], [1, ncols]],
    )


def _rev_ap(t, ncols):
    """Free-dim reversed view of a [P, ncols] tile/psum AP."""
    rs = t.ap[0][0]
    return AP(t.tensor, t.offset + ncols - 1, [[rs, t.shape[0]], [-1, ncols]])


def build_program():
    nc = bacc.Bacc(trn_type="TRN2")

    x = nc.dram_tensor("x", [S, D], MM_DT, kind="ExternalInput")
    Wq = nc.dram_tensor("Wq", [D, D], BF16, kind="ExternalInput")
    bq = nc.dram_tensor("bq", [D], F32, kind="ExternalInput")
    Wk = nc.dram_tensor("Wk", [D, D], BF16, kind="ExternalInput")
    bk = nc.dram_tensor("bk", [D], F32, kind="ExternalInput")
    Wv = nc.dram_tensor("Wv", [D, D], MM_DT, kind="ExternalInput")
    bv = nc.dram_tensor("bv", [D], F32, kind="ExternalInput")
    rel_tab = nc.dram_tensor("rel_tab", [W, D], MM_DT, kind="ExternalInput")
    Wpk = nc.dram_tensor("Wpk", [D, D], BF16, kind="ExternalInput")
    bpk = nc.dram_tensor("bpk", [D], F32, kind="ExternalInput")
    Wpq = nc.dram_tensor("Wpq", [D, D], BF16, kind="ExternalInput")
    bpq = nc.dram_tensor("bpq", [D], F32, kind="ExternalInput")
    Wo = nc.dram_tensor("Wo", [D, D], MM_DT, kind="ExternalInput")
    bo = nc.dram_tensor("bo", [D], F32, kind="ExternalInput")
    y = nc.dram_tensor("y", [S, D], F32, kind="ExternalOutput")

    with tile.TileContext(nc) as tc:
        with (
            tc.tile_pool(name="const", bufs=1) as constp,
            tc.tile_pool(name="persist", bufs=1) as persist,
        ):
            ident = constp.tile([128, 128], F32, name="ident")
            make_identity(nc, ident)
            ident_r = constp.tile([128, 128], MM_DT, name="ident_r")
            nc.scalar.copy(ident_r[:], ident[:])
            ident_b = constp.tile([128, 128], BF16, name="ident_b")
            nc.scalar.copy(ident_b[:], ident[:])

            # =========================== phase A ===========================
            with (
                tc.tile_pool(name="wload", bufs=1) as wload,
                tc.tile_pool(name="ps_xt", bufs=1, space="PSUM") as ps_xt,
                tc.tile_pool(name="ps_rt", bufs=2, space="PSUM") as ps_rt,
                tc.tile_pool(name="ps_pj", bufs=3, space="PSUM") as ps_pj,
            ):

                def load_packed(dram, nrows, name, eng, dt=MM_DT, pool=None):
                    """One big DMA: [nrows, D] row chunks packed side by side
                    in the free dim; chunk c = tile[:, c*D:(c+1)*D]."""
                    nch = (nrows + 127) // 128
                    t = (pool or wload).tile([128, nch * D], dt, name=name)
                    full = nrows // 128
                    flat = dram[:, :].rearrange("a b -> (a b)")
                    rs = t.ap[0][0]
                    if full:
                        eng.dma_start(
                            AP(t.tensor, t.offset, [[rs, 128], [D, full], [1, D]]),
                            AP(flat.tensor, 0, [[D, 128], [128 * D, full], [1, D]]),
                        )
                    if full < nch:  # remainder rows
                        p = nrows - full * 128
                        eng.dma_start(
                            t[:p, full * D : full * D + D],
                            dram[full * 128 : nrows, :],
                        )
                    return [t[:, c * D : (c + 1) * D] for c in range(nch)]

                # x first (critical path to xT), weights on the Pool/SWDGE
                # path so HWDGE stays free for latency-critical DMAs.
                x_t = load_packed(x, S, "x", eng=nc.sync)
                Wq_t = load_packed(Wq, D, "Wq", eng=nc.gpsimd, dt=BF16)
                Wk_t = load_packed(Wk, D, "Wk", eng=nc.gpsimd, dt=BF16)
                rel_t = load_packed(rel_tab, W, "rel", eng=nc.gpsimd)
                Wpk_t = load_packed(Wpk, D, "Wpk", eng=nc.gpsimd, dt=BF16)
                Wpq_t = load_packed(Wpq, D, "Wpq", eng=nc.gpsimd, dt=BF16)
                Wv_t = load_packed(Wv, D, "Wv", eng=nc.gpsimd)
                # Wo chunk c holds rows c*128..c*128+127 = head pair c;
                # lives in persist (read in phase C, after wload closes)
                Wo_h2 = load_packed(Wo, D, "Wo", eng=nc.gpsimd, pool=persist)

                bv_bc = constp.tile([128, D], F32, name="bv_bc")
                nc.sync.dma_start(bv_bc[:], AP(bv[:].tensor, 0, [[0, 128], [1, D]]))
                bo_bc = constp.tile([128, D], F32, name="bo_bc")
                nc.sync.dma_start(bo_bc[:], AP(bo[:].tensor, 0, [[0, 128], [1, D]]))

                def load_bias_cols(dram, name):
                    t = constp.tile([128, 4], F32, name=name)
                    rs = t.ap[0][0]
                    nc.sync.dma_start(
                        AP(t.tensor, t.offset, [[rs, 128], [1, 4], [1, 1]]),
                        AP(dram[:].tensor, 0, [[1, 128], [128, 4], [1, 1]]),
                    )
                    return [t[:, c : c + 1] for c in range(4)]

                bq_t = load_bias_cols(bq, "bq")
                bk_t = load_bias_cols(bk, "bk")
                bpk_t = load_bias_cols(bpk, "bpk")
                bpq_t = load_bias_cols(bpq, "bpq")

                # ---- xT via PE transpose; evicted twice: fp32r for the
                # v projection, bf16 for the q/k projections ----
                xT_t, xTb_t = [], []
                for ec in range(4):
                    ps = ps_xt.tile([128, S], F32, name="ps_xtt", tag="ps_xtt")
                    for sc in range(4):
                        nc.tensor.matmul(
                            ps[:, sc * 128 : (sc + 1) * 128].bitcast(MM_DT),
                            x_t[sc][:, ec * 128 : (ec + 1) * 128],
                            ident_r[:],
                            is_transpose=True,
                            start=(sc == 0),
                            stop=(sc == 3),
                        )
                    t = wload.tile([128, S], MM_DT, name=f"xT{ec}")
                    nc.scalar.copy(t[:], ps[:])
                    xT_t.append(t)
                    tb = wload.tile([128, S], BF16, name=f"xTb{ec}")
                    nc.vector.tensor_copy(tb[:], ps[:])
                    xTb_t.append(tb)

                # ---- qT, kT (per-partition bias) ----
                def proj_T(W_t, b_t, name):
                    out = []
                    for dcc in range(4):
                        ps = ps_pj.tile([128, S], F32, name="ps_prj", tag="ps_prj")
                        for ec in range(4):
                            nc.tensor.matmul(
                                ps[:],
                                W_t[ec][:, dcc * 128 : (dcc + 1) * 128],
                                xTb_t[ec][:],
                                start=(ec == 0),
                                stop=(ec == 3),
                            )
                        t = persist.tile([128, S], BF16, name=f"{name}{dcc}")
                        nc.scalar.activation(
                            t[:],
                            ps[:],
                            mybir.ActivationFunctionType.Identity,
                            bias=b_t[dcc],
                        )
                        out.append(t)
                    return out

                qT_t = proj_T(Wq_t, bq_t, "qT")
                kT_t = proj_T(Wk_t, bk_t, "kT")

                # ---- v straight into ones-augmented per-head layout ----
                # per sc: tile [128, 8*65]; head h: cols [65h, 65h+64) = v,
                # col 65h+64 = 1.0.  AV lhsT = tile[:, 65h:65h+65].
                vh_all = []
                for sc in range(4):
                    ps = ps_pj.tile([128, D], F32, name="ps_vv", tag="ps_prj")
                    for ec in range(4):
                        nc.tensor.matmul(
                            ps[:],
                            xT_t[ec][:, sc * 128 : (sc + 1) * 128],
                            Wv_t[ec][:],
                            start=(ec == 0),
                            stop=(ec == 3),
                        )
                    va = persist.tile([128, H * (DH + 1)], MM_DT, name=f"vaug{sc}")
                    rs = va.ap[0][0]
                    nc.vector.tensor_add(
                        AP(va.tensor, va.offset, [[rs, 128], [DH + 1, H], [1, DH]]),
                        ps[:],
                        bv_bc[:],
                    )
                    nc.vector.memset(
                        AP(va.tensor, va.offset + DH, [[rs, 128], [DH + 1, H], [1, 1]]).bitcast(F32),
                        1.0,
                    )
                    vh_all.append(va)

                def vaug_h(h, sc):
                    return vh_all[sc][:, h * (DH + 1) : (h + 1) * (DH + 1)]

                # ---- rel_tabT via PE transpose: [512, 1023] ----
                relT_t = []
                for dc in range(4):
                    ps = ps_rt.tile([128, WP], F32, name="ps_rtt", tag="ps_rtt")
                    for rc in range(8):
                        # last chunk has 127 valid rows; transpose all 128 --
                        # the garbage column lands in the pad col 1023, which
                        # the eviction below never reads.
                        nc.tensor.matmul(
                            ps[:, rc * 128 : rc * 128 + 128].bitcast(MM_DT),
                            rel_t[rc][:, dc * 128 : (dc + 1) * 128],
                            ident_r[:],
                            is_transpose=True,
                            start=(rc % 4 == 0),
                            stop=(rc % 4 == 3),
                        )
                    t = wload.tile([128, WP], BF16, name=f"relT{dc}")
                    if dc % 2 == 0:
                        nc.vector.tensor_copy(t[:, 0:W], ps[:, 0:W])
                    else:
                        nc.scalar.copy(t[:, 0:W], ps[:, 0:W])
                    nc.vector.memset(t[:, W:WP], 0.0)
                    relT_t.append(t)

                # ---- P_kT [512, 1024] and P_qT flipped ----
                def posproj_chunk(W_t, b_t, name, flip, dcc):
                    ps = ps_rt.tile([128, WP], F32, name="ps_pp", tag="ps_rtt")
                    for n0 in (0, 512):
                        for ec in range(4):
                            nc.tensor.matmul(
                                ps[:, n0 : n0 + 512],
                                W_t[ec][:, dcc * 128 : (dcc + 1) * 128],
                                relT_t[ec][:, n0 : n0 + 512],
                                start=(ec == 0),
                                stop=(ec == 3),
                            )
                    t = persist.tile([128, WP], BF16, name=f"{name}{dcc}")
                    if flip:
                        nc.scalar.activation(
                            t[:, 0:W],
                            _rev_ap(ps, W),
                            mybir.ActivationFunctionType.Identity,
                            bias=b_t[dcc],
                        )
                        nc.vector.memset(t[:, W:WP], 0.0)
                    else:
                        nc.scalar.activation(
                            t[:],
                            ps[:],
                            mybir.ActivationFunctionType.Identity,
                            bias=b_t[dcc],
                        )
                    return t

                PkT_t, PqTf_t = [], []
                for dcc in range(4):
                    PkT_t.append(
                        posproj_chunk(Wpk_t, bpk_t, "PkT", False, dcc)
                    )
                    PqTf_t.append(
                        posproj_chunk(Wpq_t, bpq_t, "PqTf", True, dcc)
                    )

            # =========================== phase B ===========================
            # Heads in pairs: even head on partitions 0-63, odd on 64-127.
            with (
                tc.tile_pool(name="hwork", bufs=2) as hwork,
                tc.tile_pool(name="ps_qp", bufs=3, space="PSUM") as ps_qp,
                tc.tile_pool(name="ps_lg", bufs=2, space="PSUM") as ps_lg,
                tc.tile_pool(name="ps_av", bufs=2, space="PSUM") as ps_av,
                tc.tile_pool(name="ps_y", bufs=1, space="PSUM") as ps_yp,
            ):

                def qp_pipeline(thT, PhT, tag, dt):
                    """qP/kPf window -> evict -> ONE merged diag read.
                    Window of qP row-chunk ic is the 640 cols
                    [384-i0, 1024-i0); computed as 384+256 col matmuls (both
                    >=256 keeps fp32r at 1 cyc/row).  All 4 chunks evict into
                    one [128, 4*640] tile; a single 3-D-AP DMA pulls the four
                    diagonals at once.  Returns the [128, 4*512] diag tile."""
                    sb = hwork.tile([128, 4 * WIN], dt, name=f"{tag}sb", bufs=3)
                    for ic in range(4):
                        i0 = ic * 128
                        pa = ps_qp.tile([128, 384], F32, name="ps_qpa", tag="ps_qp")
                        nc.tensor.matmul(
                            pa[:],
                            thT[:, i0 : i0 + 128],
                            PhT[:, 384 - i0 : 768 - i0],
                        )
                        pb = ps_qp.tile([128, 256], F32, name="ps_qpb", tag="ps_qp")
                        nc.tensor.matmul(
                            pb[:],
                            thT[:, i0 : i0 + 128],
                            PhT[:, 768 - i0 : 1024 - i0],
                        )
                        c0 = ic * WIN
                        if ic % 2 == 0:
                            nc.vector.tensor_copy(sb[:, c0 : c0 + 384], pa[:])
                            nc.scalar.copy(sb[:, c0 + 384 : c0 + 640], pb[:])
                        else:
                            nc.scalar.copy(sb[:, c0 : c0 + 384], pa[:])
                            nc.vector.tensor_copy(sb[:, c0 + 384 : c0 + 640], pb[:])
                    dg = hwork.tile([128, 4 * S], dt, name=f"{tag}dg", bufs=3)
                    nc.sync.dma_start(dg[:], _merged_diag_ap(sb, 127, 4, WIN, 128, S))
                    return dg

                outT_pair = [
                    persist.tile([128, S], MM_DT, name=f"outT{p}") for p in range(4)
                ]

                def head_views(h):
                    dc, hs = h // 2, (h % 2) * DH
                    return (
                        qT_t[dc][hs : hs + DH, :],
                        kT_t[dc][hs : hs + DH, :],
                        PkT_t[dc][hs : hs + DH, :],
                        PqTf_t[dc][hs : hs + DH, :],
                    )

                def emit_pipes(h):
                    qhT, khT, PkhT, PqhTf = head_views(h)
                    c2p = qp_pipeline(qhT, PkhT, "qp", MM_DT)
                    p2cT = qp_pipeline(khT, PqhTf, "kp", BF16)
                    return c2p, p2cT

                # ysb pre-loaded with bo; per-pair phase C accumulates
                # into it via Pool adds as soon as each pair completes
                ysb = hwork.tile([128, 4 * D], F32, name="ysb", bufs=1)
                for sc in range(4):
                    nc.vector.tensor_copy(ysb[:, sc * D : (sc + 1) * D], bo_bc[:])

                def emit_pairC_item(p, sc):
                    ps = ps_yp.tile([128, D], F32, name="ps_y", tag="ps_y")
                    nc.tensor.matmul(
                        ps[:],
                        outT_pair[p][:, sc * 128 : (sc + 1) * 128],
                        Wo_h2[p][:],
                    )
                    nc.vector.tensor_tensor(
                        ysb[:, sc * D : (sc + 1) * D],
                        ysb[:, sc * D : (sc + 1) * D],
                        ps[:],
                        op=mybir.AluOpType.add,
                    )
                    if p == 3:
                        nc.sync.dma_start(
                            y[sc * 128 : (sc + 1) * 128, :],
                            ysb[:, sc * D : (sc + 1) * D],
                        )

                horder = [0, 1, 3, 2, 5, 4, 7, 6]  # end on an even head
                pipes = {horder[0]: emit_pipes(horder[0]),
                         horder[1]: emit_pipes(horder[1])}
                pendingC = []
                for hi, h in enumerate(horder):
                    qhT, khT, PkhT, PqhTf = head_views(h)
                    if hi + 2 < H:
                        pipes[horder[hi + 2]] = emit_pipes(horder[hi + 2])
                    c2p, p2cT = pipes.pop(h)

                    ex = []
                    for jc in range(4):
                        ps = ps_lg.tile([128, S], F32, name="ps_lg", tag="ps_lg")
                        nc.tensor.matmul(
                            ps[:],
                            khT[:, jc * 128 : (jc + 1) * 128],
                            qhT[:],
                            start=True,
                            stop=False,
                        )
                        for ic in range(4):
                            nc.tensor.matmul(
                                ps[:, ic * 128 : (ic + 1) * 128].bitcast(MM_DT),
                                c2p[:, ic * S + jc * 128 : ic * S + (jc + 1) * 128],
                                ident_r[:],
                                is_transpose=True,
                                start=False,
                                stop=False,
                            )
                        nc.tensor.matmul(
                            ps[:],
                            ident_b[:],
                            p2cT[:, jc * S : (jc + 1) * S],
                            start=False,
                            stop=True,
                        )
                        et = hwork.tile([128, S], MM_DT, name=f"ex{jc}", bufs=3)
                        nc.scalar.activation(
                            et[:],
                            ps[:],
                            mybir.ActivationFunctionType.Exp,
                            scale=SCALE,
                        )
                        ex.append(et[:])
                        if pendingC:
                            emit_pairC_item(*pendingC.pop(0))

                    # AV with ones-row: denominator lands in psum row 64.
                    avt = ps_av.tile([DH + 1, S], F32, name="ps_avt", tag="ps_avt")
                    for jc in range(4):
                        nc.tensor.matmul(
                            avt[:],
                            vaug_h(h, jc),
                            ex[jc],
                            start=(jc == 0),
                            stop=(jc == 3),
                        )
                    zr = hwork.tile([DH + 1, S], F32, name="zrow", bufs=2)
                    nc.vector.reciprocal(zr[DH : DH + 1, :], avt[DH : DH + 1, :])
                    rbc = hwork.tile([DH, S], F32, name="rbc", bufs=2)
                    rs_zr = zr.ap[0][0]
                    nc.gpsimd.partition_broadcast(
                        rbc[:],
                        AP(zr.tensor, zr.offset + DH * rs_zr, [[rs_zr, 1], [1, S]]),
                    )
                    if h % 2 == 0:
                        nc.vector.tensor_mul(
                            outT_pair[h // 2][0:DH, :], avt[0:DH, :], rbc[:]
                        )
                    else:
                        # odd head: normalize into a staging tile, then DMA
                        # into partitions 64..127 of the pair tile (engines
                        # cannot shift partitions; DMA can)
                        omt = hwork.tile([DH, S], MM_DT, name="otmp", bufs=2)
                        nc.vector.tensor_mul(omt[:], avt[0:DH, :], rbc[:])
                        nc.sync.dma_start(
                            outT_pair[h // 2][DH:128, :], omt[:]
                        )
                    if hi % 2 == 1:
                        pendingC.extend((h // 2, sc) for sc in range(4))

                while pendingC:
                    emit_pairC_item(*pendingC.pop(0))

    nc.compile()
    return nc


_cache_lock = threading.Lock()
_cached_nc = None


def _get_program():
    global _cached_nc
    with _cache_lock:
        if _cached_nc is None:
            _cached_nc = build_program()
    return _cached_nc


def kernel(**inputs):
    x = np.ascontiguousarray(np.asarray(inputs["x"], dtype=np.float32))
    B = x.shape[0]
    assert x.shape == (B, S, D)

    weights = {
        k: np.ascontiguousarray(np.asarray(inputs[k], dtype=np.float32))
        for k in (
            "Wq", "bq", "Wk", "bk", "Wv", "bv",
            "rel_tab", "bpk", "bpq", "Wo", "bo",
        )
    }
    for k in ("Wpk", "Wpq", "Wq", "Wk"):  # device expects bf16 here
        weights[k] = np.ascontiguousarray(
            np.asarray(inputs[k], dtype=np.float32).astype(ml_dtypes.bfloat16)
        )

    nc = _get_program()
    in_maps = [{"x": x[c], **weights} for c in range(NCORES)]
    res = run_bass_kernel_spmd(nc, in_maps, core_ids=list(range(NCORES)))
    out = np.stack([res.results[c]["y"] for c in range(NCORES)], axis=0)
    return out.astype(np.float32)


if __name__ == "__main__":
    rng = np.random.default_rng(0)
    ins = {
        "x": rng.standard_normal((NCORES, S, D), dtype=np.float32),
        "rel_tab": rng.standard_normal((W, D), dtype=np.float32),
    }
    for nm in ("Wq", "Wk", "Wv", "Wpk", "Wpq", "Wo"):
        ins[nm] = rng.standard_normal((D, D), dtype=np.float32) * 0.04
    for nm in ("bq", "bk", "bv", "bpk", "bpq", "bo"):
        ins[nm] = rng.standard_normal(D).astype(np.float32) * 0.01
    out = kernel(**ins)
    print("ran:", out.shape, out.dtype, np.abs(out).max())


# revision 28
# speedup vs baseline: 1.0586x; 1.0324x over previous
"""Disentangled multi-head attention (DeBERTa-style) Trainium2 Bass kernel.

Full inputs in, full outputs out. Sharding: batch (B=8) across 8 cores, data
parallel; each core computes all H=8 heads for its batch element.

Math (per batch b):
  q,k,v = x@W? + b?                                   [S, D]
  rel_emb[i,j] = rel_tab[j-i+511]  (Toeplitz: only 1023 distinct rows)
  P_k = rel_tab@Wpk + bpk ; P_q = rel_tab@Wpq + bpq   [1023, D]
  c2c[i,j] = q_i . k_j
  c2p[i,j] = q_i . P_k[j-i+511]  = qP[i, j-i+511],    qP  = q @ P_k^T
  p2c[i,j] = k_j . P_q[j-i+511]  = kPf[j, i-j+511],   kPf = k @ P_qflip^T
  out = softmax((c2c+c2p+p2c)/sqrt(3*64)) @ v ; y = out@Wo + bo

Kernel works in transposed-logits layout logitsT[j, i]:
  c2cT  : matmul(lhsT=khT_chunk, rhs=qhT)
  c2pT  : diag-DMA qP rows (per-partition shifted slice) then PE-transpose
  p2cT  : diag-DMA kPf rows directly (already [j, i])
  softmax: exp on ACT; denominator via ones-column in the AV matmul;
  normalize after AV.

Perf structure (cost-model driven):
  - inputs loaded with one packed DMA per tensor, issued from the Pool
    engine's SWDGE path so descriptor-gen doesn't serialize on HWDGE
  - qP/kPf windows computed as 384+256 col matmuls (both >=256 so fp32r
    streams 1 cyc/row; 512+128 pays 4x on the 128)
  - the 4 per-chunk diagonal reads of each pipeline merge into ONE 3-D-AP
    DMA (HWDGE descriptor-gen is ~630ns per DMA instruction, serialized)
  - v is evicted directly in ones-augmented per-head layout; odd heads use
    a reversed [1|v] layout so their AV matmul lands on partitions 63..127,
    letting head pairs share one [128, S] outT tile
  - phase C contracts head PAIRS in single K=128 matmuls (16 not 32)
"""

import math
import os
import sys
import threading

import numpy as np
import ml_dtypes

for _p in ("/opt/trn_rl_repo",):
    if _p not in sys.path and os.path.isdir(_p):
        sys.path.insert(0, _p)

import concourse.bacc as bacc
import concourse.bass as bass
import concourse.mybir as mybir
import concourse.tile as tile
from concourse.ap import AP
from concourse.bass_utils import run_bass_kernel_spmd
from concourse.masks import make_identity

S = 512
D = 512
H = 8
DH = 64
L = 512
W = 2 * L - 1  # 1023
WP = 1024  # padded so fp32r matmuls keep even 512-wide moving dims
WIN = 640  # 639-wide diag window, rounded up
NCORES = 8
SCALE = 1.0 / math.sqrt(3.0 * DH)

F32 = mybir.dt.float32
F32R = mybir.dt.float32r
BF16 = mybir.dt.bfloat16
MM_DT = F32R


def _merged_diag_ap(t, col0, nchunks, chunk_stride, nrows, ncols):
    """Per-partition shifted read over nchunks windows packed in one tile:
    out[p, c*ncols + j] = t[p, c*chunk_stride + col0 - p + j]."""
    rs = t.ap[0][0]
    return AP(
        t.tensor,
        t.offset + col0,
        [[rs - 1, nrows], [chunk_stride, nchunks], [1, ncols]],
    )


def _rev_ap(t, ncols):
    """Free-dim reversed view of a [P, ncols] tile/psum AP."""
    rs = t.ap[0][0]
    return AP(t.tensor, t.offset + ncols - 1, [[rs, t.shape[0]], [-1, ncols]])


def build_program():
    nc = bacc.Bacc(trn_type="TRN2")

    x = nc.dram_tensor("x", [S, D], MM_DT, kind="ExternalInput")
    Wq = nc.dram_tensor("Wq", [D, D], MM_DT, kind="ExternalInput")
    bq = nc.dram_tensor("bq", [D], F32, kind="ExternalInput")
    Wk = nc.dram_tensor("Wk", [D, D], MM_DT, kind="ExternalInput")
    bk = nc.dram_tensor("bk", [D], F32, kind="ExternalInput")
    Wv = nc.dram_tensor("Wv", [D, D], MM_DT, kind="ExternalInput")
    bv = nc.dram_tensor("bv", [D], F32, kind="ExternalInput")
    rel_tab = nc.dram_tensor("rel_tab", [W, D], MM_DT, kind="ExternalInput")
    Wpk = nc.dram_tensor("Wpk", [D, D], BF16, kind="ExternalInput")
    bpk = nc.dram_tensor("bpk", [D], F32, kind="ExternalInput")
    Wpq = nc.dram_tensor("Wpq", [D, D], BF16, kind="ExternalInput")
    bpq = nc.dram_tensor("bpq", [D], F32, kind="ExternalInput")
    Wo = nc.dram_tensor("Wo", [D, D], MM_DT, kind="ExternalInput")
    bo = nc.dram_tensor("bo", [D], F32, kind="ExternalInput")
    y = nc.dram_tensor("y", [S, D], F32, kind="ExternalOutput")

    with tile.TileContext(nc) as tc:
        with (
            tc.tile_pool(name="const", bufs=1) as constp,
            tc.tile_pool(name="persist", bufs=1) as persist,
        ):
            ident = constp.tile([128, 128], F32, name="ident")
            make_identity(nc, ident)
            ident_r = constp.tile([128, 128], MM_DT, name="ident_r")
            nc.scalar.copy(ident_r[:], ident[:])
            ident_b = constp.tile([128, 128], BF16, name="ident_b")
            nc.scalar.copy(ident_b[:], ident[:])

            # =========================== phase A ===========================
            with (
                tc.tile_pool(name="wload", bufs=1) as wload,
                tc.tile_pool(name="ps_xt", bufs=1, space="PSUM") as ps_xt,
                tc.tile_pool(name="ps_rt", bufs=2, space="PSUM") as ps_rt,
                tc.tile_pool(name="ps_pj", bufs=3, space="PSUM") as ps_pj,
            ):

                def load_packed(dram, nrows, name, eng, dt=MM_DT, pool=None):
                    """One big DMA: [nrows, D] row chunks packed side by side
                    in the free dim; chunk c = tile[:, c*D:(c+1)*D]."""
                    nch = (nrows + 127) // 128
                    t = (pool or wload).tile([128, nch * D], dt, name=name)
                    full = nrows // 128
                    flat = dram[:, :].rearrange("a b -> (a b)")
                    rs = t.ap[0][0]
                    if full:
                        eng.dma_start(
                            AP(t.tensor, t.offset, [[rs, 128], [D, full], [1, D]]),
                            AP(flat.tensor, 0, [[D, 128], [128 * D, full], [1, D]]),
                        )
                    if full < nch:  # remainder rows
                        p = nrows - full * 128
                        eng.dma_start(
                            t[:p, full * D : full * D + D],
                            dram[full * 128 : nrows, :],
                        )
                    return [t[:, c * D : (c + 1) * D] for c in range(nch)]

                # x first (critical path to xT), weights on the Pool/SWDGE
                # path so HWDGE stays free for latency-critical DMAs.
                x_t = load_packed(x, S, "x", eng=nc.sync)
                Wq_t = load_packed(Wq, D, "Wq", eng=nc.gpsimd)
                Wk_t = load_packed(Wk, D, "Wk", eng=nc.gpsimd)
                rel_t = load_packed(rel_tab, W, "rel", eng=nc.gpsimd)
                Wpk_t = load_packed(Wpk, D, "Wpk", eng=nc.gpsimd, dt=BF16)
                Wpq_t = load_packed(Wpq, D, "Wpq", eng=nc.gpsimd, dt=BF16)
                Wv_t = load_packed(Wv, D, "Wv", eng=nc.gpsimd)
                # Wo chunk c holds rows c*128..c*128+127 = head pair c;
                # lives in persist (read in phase C, after wload closes)
                Wo_h2 = load_packed(Wo, D, "Wo", eng=nc.gpsimd, pool=persist)

                bv_bc = constp.tile([128, D], F32, name="bv_bc")
                nc.sync.dma_start(bv_bc[:], AP(bv[:].tensor, 0, [[0, 128], [1, D]]))
                bo_bc = constp.tile([128, D], F32, name="bo_bc")
                nc.sync.dma_start(bo_bc[:], AP(bo[:].tensor, 0, [[0, 128], [1, D]]))

                def load_bias_cols(dram, name):
                    t = constp.tile([128, 4], F32, name=name)
                    rs = t.ap[0][0]
                    nc.sync.dma_start(
                        AP(t.tensor, t.offset, [[rs, 128], [1, 4], [1, 1]]),
                        AP(dram[:].tensor, 0, [[1, 128], [128, 4], [1, 1]]),
                    )
                    return [t[:, c : c + 1] for c in range(4)]

                bq_t = load_bias_cols(bq, "bq")
                bk_t = load_bias_cols(bk, "bk")
                bpk_t = load_bias_cols(bpk, "bpk")
                bpq_t = load_bias_cols(bpq, "bpq")

                # ---- xT via PE transpose ----
                xT_t = []
                for ec in range(4):
                    ps = ps_xt.tile([128, S], F32, name="ps_xtt", tag="ps_xtt")
                    for sc in range(4):
                        nc.tensor.matmul(
                            ps[:, sc * 128 : (sc + 1) * 128].bitcast(MM_DT),
                            x_t[sc][:, ec * 128 : (ec + 1) * 128],
                            ident_r[:],
                            is_transpose=True,
                            start=(sc == 0),
                            stop=(sc == 3),
                        )
                    t = wload.tile([128, S], MM_DT, name=f"xT{ec}")
                    nc.scalar.copy(t[:], ps[:])
                    xT_t.append(t)

                # ---- qT, kT (per-partition bias) ----
                def proj_T(W_t, b_t, name):
                    out = []
                    for dcc in range(4):
                        ps = ps_pj.tile([128, S], F32, name="ps_prj", tag="ps_prj")
                        for ec in range(4):
                            nc.tensor.matmul(
                                ps[:],
                                W_t[ec][:, dcc * 128 : (dcc + 1) * 128],
                                xT_t[ec][:],
                                start=(ec == 0),
                                stop=(ec == 3),
                            )
                        t = persist.tile([128, S], BF16, name=f"{name}{dcc}")
                        nc.scalar.activation(
                            t[:],
                            ps[:],
                            mybir.ActivationFunctionType.Identity,
                            bias=b_t[dcc],
                        )
                        out.append(t)
                    return out

                qT_t = proj_T(Wq_t, bq_t, "qT")
                kT_t = proj_T(Wk_t, bk_t, "kT")

                # ---- v straight into ones-augmented per-head layout ----
                # per sc: tile [128, 8*65]; head h: cols [65h, 65h+64) = v,
                # col 65h+64 = 1.0.  AV lhsT = tile[:, 65h:65h+65].
                vh_all = []
                for sc in range(4):
                    ps = ps_pj.tile([128, D], F32, name="ps_vv", tag="ps_prj")
                    for ec in range(4):
                        nc.tensor.matmul(
                            ps[:],
                            xT_t[ec][:, sc * 128 : (sc + 1) * 128],
                            Wv_t[ec][:],
                            start=(ec == 0),
                            stop=(ec == 3),
                        )
                    va = persist.tile([128, H * (DH + 1)], MM_DT, name=f"vaug{sc}")
                    rs = va.ap[0][0]
                    nc.vector.tensor_add(
                        AP(va.tensor, va.offset, [[rs, 128], [DH + 1, H], [1, DH]]),
                        ps[:],
                        bv_bc[:],
                    )
                    nc.vector.memset(
                        AP(va.tensor, va.offset + DH, [[rs, 128], [DH + 1, H], [1, 1]]).bitcast(F32),
                        1.0,
                    )
                    vh_all.append(va)

                def vaug_h(h, sc):
                    return vh_all[sc][:, h * (DH + 1) : (h + 1) * (DH + 1)]

                # ---- rel_tabT via PE transpose: [512, 1023] ----
                relT_t = []
                for dc in range(4):
                    ps = ps_rt.tile([128, WP], F32, name="ps_rtt", tag="ps_rtt")
                    for rc in range(8):
                        # last chunk has 127 valid rows; transpose all 128 --
                        # the garbage column lands in the pad col 1023, which
                        # the eviction below never reads.
                        nc.tensor.matmul(
                            ps[:, rc * 128 : rc * 128 + 128].bitcast(MM_DT),
                            rel_t[rc][:, dc * 128 : (dc + 1) * 128],
                            ident_r[:],
                            is_transpose=True,
                            start=(rc % 4 == 0),
                            stop=(rc % 4 == 3),
                        )
                    t = wload.tile([128, WP], BF16, name=f"relT{dc}")
                    if dc % 2 == 0:
                        nc.vector.tensor_copy(t[:, 0:W], ps[:, 0:W])
                    else:
                        nc.scalar.copy(t[:, 0:W], ps[:, 0:W])
                    nc.vector.memset(t[:, W:WP], 0.0)
                    relT_t.append(t)

                # ---- P_kT [512, 1024] and P_qT flipped ----
                def posproj_chunk(W_t, b_t, name, flip, dcc):
                    ps = ps_rt.tile([128, WP], F32, name="ps_pp", tag="ps_rtt")
                    for n0 in (0, 512):
                        for ec in range(4):
                            nc.tensor.matmul(
                                ps[:, n0 : n0 + 512],
                                W_t[ec][:, dcc * 128 : (dcc + 1) * 128],
                                relT_t[ec][:, n0 : n0 + 512],
                                start=(ec == 0),
                                stop=(ec == 3),
                            )
                    t = persist.tile([128, WP], BF16, name=f"{name}{dcc}")
                    if flip:
                        nc.scalar.activation(
                            t[:, 0:W],
                            _rev_ap(ps, W),
                            mybir.ActivationFunctionType.Identity,
                            bias=b_t[dcc],
                        )
                        nc.vector.memset(t[:, W:WP], 0.0)
                    else:
                        nc.scalar.activation(
                            t[:],
                            ps[:],
                            mybir.ActivationFunctionType.Identity,
                            bias=b_t[dcc],
                        )
                    return t

                PkT_t, PqTf_t = [], []
                for dcc in range(4):
                    PkT_t.append(
                        posproj_chunk(Wpk_t, bpk_t, "PkT", False, dcc)
                    )
                    PqTf_t.append(
                        posproj_chunk(Wpq_t, bpq_t, "PqTf", True, dcc)
                    )

            # =========================== phase B ===========================
            # Heads in pairs: even head on partitions 0-63, odd on 64-127.
            with (
                tc.tile_pool(name="hwork", bufs=2) as hwork,
                tc.tile_pool(name="ps_qp", bufs=3, space="PSUM") as ps_qp,
                tc.tile_pool(name="ps_lg", bufs=2, space="PSUM") as ps_lg,
                tc.tile_pool(name="ps_av", bufs=2, space="PSUM") as ps_av,
                tc.tile_pool(name="ps_y", bufs=1, space="PSUM") as ps_yp,
            ):

                def qp_pipeline(thT, PhT, tag, dt):
                    """qP/kPf window -> evict -> ONE merged diag read.
                    Window of qP row-chunk ic is the 640 cols
                    [384-i0, 1024-i0); computed as 384+256 col matmuls (both
                    >=256 keeps fp32r at 1 cyc/row).  All 4 chunks evict into
                    one [128, 4*640] tile; a single 3-D-AP DMA pulls the four
                    diagonals at once.  Returns the [128, 4*512] diag tile."""
                    sb = hwork.tile([128, 4 * WIN], dt, name=f"{tag}sb", bufs=3)
                    for ic in range(4):
                        i0 = ic * 128
                        pa = ps_qp.tile([128, 384], F32, name="ps_qpa", tag="ps_qp")
                        nc.tensor.matmul(
                            pa[:],
                            thT[:, i0 : i0 + 128],
                            PhT[:, 384 - i0 : 768 - i0],
                        )
                        pb = ps_qp.tile([128, 256], F32, name="ps_qpb", tag="ps_qp")
                        nc.tensor.matmul(
                            pb[:],
                            thT[:, i0 : i0 + 128],
                            PhT[:, 768 - i0 : 1024 - i0],
                        )
                        c0 = ic * WIN
                        if ic % 2 == 0:
                            nc.vector.tensor_copy(sb[:, c0 : c0 + 384], pa[:])
                            nc.scalar.copy(sb[:, c0 + 384 : c0 + 640], pb[:])
                        else:
                            nc.scalar.copy(sb[:, c0 : c0 + 384], pa[:])
                            nc.vector.tensor_copy(sb[:, c0 + 384 : c0 + 640], pb[:])
                    dg = hwork.tile([128, 4 * S], dt, name=f"{tag}dg", bufs=3)
                    nc.sync.dma_start(dg[:], _merged_diag_ap(sb, 127, 4, WIN, 128, S))
                    return dg

                outT_pair = [
                    persist.tile([128, S], MM_DT, name=f"outT{p}") for p in range(4)
                ]

                def head_views(h):
                    dc, hs = h // 2, (h % 2) * DH
                    return (
                        qT_t[dc][hs : hs + DH, :],
                        kT_t[dc][hs : hs + DH, :],
                        PkT_t[dc][hs : hs + DH, :],
                        PqTf_t[dc][hs : hs + DH, :],
                    )

                def emit_pipes(h):
                    qhT, khT, PkhT, PqhTf = head_views(h)
                    c2p = qp_pipeline(qhT, PkhT, "qp", MM_DT)
                    p2cT = qp_pipeline(khT, PqhTf, "kp", BF16)
                    return c2p, p2cT

                # ysb pre-loaded with bo; per-pair phase C accumulates
                # into it via Pool adds as soon as each pair completes
                ysb = hwork.tile([128, 4 * D], F32, name="ysb", bufs=1)
                for sc in range(4):
                    nc.vector.tensor_copy(ysb[:, sc * D : (sc + 1) * D], bo_bc[:])

                def emit_pairC_item(p, sc):
                    ps = ps_yp.tile([128, D], F32, name="ps_y", tag="ps_y")
                    nc.tensor.matmul(
                        ps[:],
                        outT_pair[p][:, sc * 128 : (sc + 1) * 128],
                        Wo_h2[p][:],
                    )
                    nc.vector.tensor_tensor(
                        ysb[:, sc * D : (sc + 1) * D],
                        ysb[:, sc * D : (sc + 1) * D],
                        ps[:],
                        op=mybir.AluOpType.add,
                    )
                    if p == 3:
                        nc.sync.dma_start(
                            y[sc * 128 : (sc + 1) * 128, :],
                            ysb[:, sc * D : (sc + 1) * D],
                        )

                horder = [0, 1, 3, 2, 5, 4, 7, 6]  # end on an even head
                pipes = {horder[0]: emit_pipes(horder[0]),
                         horder[1]: emit_pipes(horder[1])}
                pendingC = []
                for hi, h in enumerate(horder):
                    qhT, khT, PkhT, PqhTf = head_views(h)
                    if hi + 2 < H:
                        pipes[horder[hi + 2]] = emit_pipes(horder[hi + 2])
                    c2p, p2cT = pipes.pop(h)

                    ex = []
                    for jc in range(4):
                        ps = ps_lg.tile([128, S], F32, name="ps_lg", tag="ps_lg")
                        nc.tensor.matmul(
                            ps[:],
                            khT[:, jc * 128 : (jc + 1) * 128],
                            qhT[:],
                            start=True,
                            stop=False,
                        )
                        for ic in range(4):
                            nc.tensor.matmul(
                                ps[:, ic * 128 : (ic + 1) * 128].bitcast(MM_DT),
                                c2p[:, ic * S + jc * 128 : ic * S + (jc + 1) * 128],
                                ident_r[:],
                                is_transpose=True,
                                start=False,
                                stop=False,
                            )
                        nc.tensor.matmul(
                            ps[:],
                            ident_b[:],
                            p2cT[:, jc * S : (jc + 1) * S],
                            start=False,
                            stop=True,
                        )
                        et = hwork.tile([128, S], MM_DT, name=f"ex{jc}", bufs=3)
                        nc.scalar.activation(
                            et[:],
                            ps[:],
                            mybir.ActivationFunctionType.Exp,
                            scale=SCALE,
                        )
                        ex.append(et[:])
                        if pendingC:
                            emit_pairC_item(*pendingC.pop(0))

                    # AV with ones-row: denominator lands in psum row 64.
                    avt = ps_av.tile([DH + 1, S], F32, name="ps_avt", tag="ps_avt")
                    for jc in range(4):
                        nc.tensor.matmul(
                            avt[:],
                            vaug_h(h, jc),
                            ex[jc],
                            start=(jc == 0),
                            stop=(jc == 3),
                        )
                    zr = hwork.tile([DH + 1, S], F32, name="zrow", bufs=2)
                    nc.vector.reciprocal(zr[DH : DH + 1, :], avt[DH : DH + 1, :])
                    rbc = hwork.tile([DH, S], F32, name="rbc", bufs=2)
                    rs_zr = zr.ap[0][0]
                    nc.gpsimd.partition_broadcast(
                        rbc[:],
                        AP(zr.tensor, zr.offset + DH * rs_zr, [[rs_zr, 1], [1, S]]),
                    )
                    if h % 2 == 0:
                        nc.vector.tensor_mul(
                            outT_pair[h // 2][0:DH, :], avt[0:DH, :], rbc[:]
                        )
                    else:
                        # odd head: normalize into a staging tile, then DMA
                        # into partitions 64..127 of the pair tile (engines
                        # cannot shift partitions; DMA can)
                        omt = hwork.tile([DH, S], MM_DT, name="otmp", bufs=2)
                        nc.vector.tensor_mul(omt[:], avt[0:DH, :], rbc[:])
                        nc.sync.dma_start(
                            outT_pair[h // 2][DH:128, :], omt[:]
                        )
                    if hi % 2 == 1:
                        pendingC.extend((h // 2, sc) for sc in range(4))

                while pendingC:
                    emit_pairC_item(*pendingC.pop(0))

    nc.compile()
    return nc


_cache_lock = threading.Lock()
_cached_nc = None


def _get_program():
    global _cached_nc
    with _cache_lock:
        if _cached_nc is None:
            _cached_nc = build_program()
    return _cached_nc


def kernel(**inputs):
    x = np.ascontiguousarray(np.asarray(inputs["x"], dtype=np.float32))
    B = x.shape[0]
    assert x.shape == (B, S, D)

    weights = {
        k: np.ascontiguousarray(np.asarray(inputs[k], dtype=np.float32))
        for k in (
            "Wq", "bq", "Wk", "bk", "Wv", "bv",
            "rel_tab", "bpk", "bpq", "Wo", "bo",
        )
    }
    for k in ("Wpk", "Wpq"):  # device expects bf16 here
        weights[k] = np.ascontiguousarray(
            np.asarray(inputs[k], dtype=np.float32).astype(ml_dtypes.bfloat16)
        )

    nc = _get_program()
    in_maps = [{"x": x[c], **weights} for c in range(NCORES)]
    res = run_bass_kernel_spmd(nc, in_maps, core_ids=list(range(NCORES)))
    out = np.stack([res.results[c]["y"] for c in range(NCORES)], axis=0)
    return out.astype(np.float32)


if __name__ == "__main__":
    rng = np.random.default_rng(0)
    ins = {
        "x": rng.standard_normal((NCORES, S, D), dtype=np.float32),
        "rel_tab": rng.standard_normal((W, D), dtype=np.float32),
    }
    for nm in ("Wq", "Wk", "Wv", "Wpk", "Wpq", "Wo"):
        ins[nm] = rng.standard_normal((D, D), dtype=np.float32) * 0.04
    for nm in ("bq", "bk", "bv", "bpk", "bpq", "bo"):
        ins[nm] = rng.standard_normal(D).astype(np.float32) * 0.01
    out = kernel(**ins)
    print("ran:", out.shape, out.dtype, np.abs(out).max())
